# revision 27
# baseline (speedup 1.0000x reference)
"""Fused transformer block (LN + fused QKV/FF proj + MQA attention + SwiGLU FF)
on 8 TRN2 NeuronCores.

Sharding: hybrid DP2 x TP4.
  core c -> batch b = c//4, tensor-parallel shard s = c%4.
  Each core handles its batch's full 2048 tokens in feature-major layout:
    - q: 2 of 8 heads (cols 128*s .. 128*s+128 of the q block)
    - k/v: replicated (width 64 each)
    - ff: 1024 of 4096 cols of both ff_x and gate
    - attn_out / ff_out: matching row shards -> partial [1024, 2048] outputs
  Host sums the 4 partial outputs per batch (row-parallel reduction).

Device layout is feature-major (features on SBUF partitions, tokens on the
free dim) so every matmul contraction is over the partition dim.  gamma and
the q-scale (dim_head**-0.5) are folded into w_fused on the host; per-token
LayerNorm mu/rstd are computed on device via ones-vector matmuls, broadcast
across partitions through a DRAM bounce, mean-subtract applied in place on
x^T, and rstd folded into each projection's PSUM evacuation.
"""

import numpy as np
import ml_dtypes

# ---- problem shapes (hardcoded) ----
B, N, D = 2, 2048, 1024
DH = 64
HEADS = 8
ATTN_INNER = HEADS * DH          # 512
FF_INNER = 4 * D                 # 4096
T = N                            # tokens per core
P = 128
TS = 512
NTS = T // TS                    # 4
NK = D // P                      # 8
NCORES = 8
TP = 4
LH = HEADS // TP                 # 2 local heads
FF_SH = FF_INNER // TP           # 1024
FSH = LH * DH + 2 * DH + 2 * FF_SH   # 2304 packed proj cols per core
NF = FSH // P                    # 18
NKT = T // P                     # 16 key-token tiles

_BF16 = ml_dtypes.bfloat16
_F8 = ml_dtypes.float8_e4m3

_STATE = {}


def _build_nc():
    from concourse import bacc
    import concourse.tile as tile
    from concourse.tile import add_dep_helper
    import concourse.mybir as mybir

    bf16 = mybir.dt.bfloat16
    f32 = mybir.dt.float32
    f8 = mybir.dt.float8e4
    DR = mybir.MatmulPerfMode.DoubleRow
    AF = mybir.ActivationFunctionType

    nc = bacc.Bacc("TRN2", target_bir_lowering=False, debug=False)

    # x and the fused weights come as 2-level fp8 (hi + residual lo), packed
    # in DoubleRow k-pair layout [p, kp, slot, col]; wf is scaled 16x on the
    # host so the fp8 residual sits in normal range (evac divides by 16)
    xhi_d = nc.dram_tensor("xhi", [P, NK // 2, 2, T], f8, kind="ExternalInput")
    xlo_d = nc.dram_tensor("xlo", [P, NK // 2, 2, T], f8, kind="ExternalInput")
    whi_d = nc.dram_tensor("whi", [P, NK // 2, 2, FSH], f8,
                           kind="ExternalInput")
    wlo_d = nc.dram_tensor("wlo", [P, NK // 2, 2, FSH], f8,
                           kind="ExternalInput")
    sw_d = nc.dram_tensor("sw", [1, 2, FSH], f8, kind="ExternalInput")
    wao_d = nc.dram_tensor("wao", [64, 2 * D], f8, kind="ExternalInput")
    wfo_d = nc.dram_tensor("wfo", [D, D], bf16, kind="ExternalInput")
    yT_d = nc.dram_tensor("yT", [D, T], f32, kind="ExternalOutput")
    # DRAM bounce row for partition-broadcast
    rstd_d = nc.dram_tensor("rstd_bounce", [1, T], f32)

    with tile.TileContext(nc) as tc:
        with (
            tc.tile_pool(name="cp", bufs=1) as cp,
            tc.tile_pool(name="wp", bufs=1) as wp,
            tc.tile_pool(name="acts", bufs=1) as acts,
            tc.tile_pool(name="rows", bufs=1) as rows,
            tc.tile_pool(name="tmp", bufs=3) as tmp,
            tc.tile_pool(name="esp", bufs=20) as esp,
            tc.tile_pool(name="ps", bufs=1, space="PSUM") as ps,
        ):
            # ---- constants ----
            ones_col = cp.tile([P, 1], bf16)
            nc.vector.memset(ones_col, 1.0)
            ones_row_bf = cp.tile([1, P], bf16)
            nc.vector.memset(ones_row_bf, 1.0)
            f32r = mybir.dt.float32r
            ones_hi = cp.tile([P, 64], f32r)
            nc.vector.memset(ones_hi.bitcast(f32), 1.0)
            ones_row_r = cp.tile([1, P], f32r)
            nc.vector.memset(ones_row_r.bitcast(f32), 1.0)
            eps_t = cp.tile([1, 1], f32)
            nc.vector.memset(eps_t, 1e-5)
            zero_col = cp.tile([P, 1], f32)
            nc.vector.memset(zero_col, 0.0)
            neg2_col = cp.tile([P, 1], f32)
            nc.vector.memset(neg2_col, -4.0)
            # DoubleRow lhsT outer stride must be even + 16B aligned
            ones2_f8 = cp.tile([P, 2, 16], f8)
            nc.vector.memset(ones2_f8, 1.0)
            # warm the ACT function tables before the bulk DMAs queue up:
            # lazy table loads otherwise serialize behind ~9MB of weight
            # traffic and stall the first LayerNorm square by ~8us
            warm_o = cp.tile([1, 4], f32)
            for wi, fn in enumerate((AF.Silu, AF.Exp, AF.Sqrt)):
                nc.scalar.activation(warm_o[0:1, wi:wi + 1],
                                     eps_t, fn, bias=zero_col[0:1, :])

            # ---- persistent activations ----
            # q/k in fp8 DoubleRow layout: partition p holds head-dims p and
            # 32+p in slots 0/1 of the middle axis (K = 2x32 per sim matmul)
            q8 = [acts.tile([32, 2, T], f8, name=f"q8_{h}") for h in range(LH)]
            k8 = acts.tile([32, 2, T], f8)
            kv_sb = acts.tile([P, T], bf16)    # k rows 0-63, v rows 64-127
            h_sb = [acts.tile([P, T], bf16, name=f"h{j}") for j in range(NK)]
            # attention out fp8: head h in slot h (partitions 0-63), scaled
            # by 16 to sit in fp8e4 normal range (wao carries the 1/16)
            out_sb = acts.tile([64, 2, T], f8)
            rstd_b = acts.tile([P, T], f32)    # rstd broadcast
            # mu as a 1-row fp8 DoubleRow operand (slot 1 stays zero); the
            # -s_w * mu mean-correction rides the proj PSUM chain, so x is
            # never centered and mu is never partition-broadcast
            mu2_row = acts.tile([1, 2, T], f8)
            nc.vector.memset(mu2_row[:, 1, :], 0.0)
            # v in fp8 DoubleRow pairs: slot i = key-tile 2m+i, col 64 = ones
            # (softmax denominator row)
            v_aug = [acts.tile([P, 72], bf16, name=f"va{kt}") for kt in range(NKT)]
            v8 = [acts.tile([P, 2, 80], f8, name=f"v8_{m}")
                  for m in range(NKT // 2)]
            for kt in range(NKT):
                nc.vector.memset(v_aug[kt][:, 64:65], 1.0)
            for m in range(NKT // 2):
                nc.vector.memset(v8[m][:, 0, 64:65], 1.0)
                nc.vector.memset(v8[m][:, 1, 64:65], 1.0)

            # sims + exp are emitted eagerly during the projection phase
            # (slot b's sims only need slices <= b evacuated): the ACT engine
            # idles during proj and saturates on exp otherwise, so buffering
            # es pairs here converts the ACT-bound attention phase into a
            # short PE-bound drain
            es_store = {}
            pavs = {}

            def av_mm(tsq, h, m):
                if m == 0:
                    pavs[(tsq, h)] = ps.tile([P, TS], f32, tag="pav",
                                             bufs=3, name=f"pav{tsq}_{h}")
                nc.tensor.matmul(
                    pavs[(tsq, h)][0:65, :], lhsT=v8[m][:, :, 0:65],
                    rhs=es_store.pop((tsq, h, m)),
                    start=(m == 0), stop=(m == NKT // 2 - 1),
                    perf_mode=DR)

            def emit_sims_pair(tsq, kt):
                qcol = slice(tsq * TS, (tsq + 1) * TS)
                kcols = slice(kt * P, (kt + 1) * P)
                for h in range(LH):
                    psim = ps.tile([P, TS], f32, tag="pp", bufs=5,
                                   name=f"psim{tsq}_{h}_{kt}")
                    nc.tensor.matmul(psim, lhsT=k8[:, :, kcols],
                                     rhs=q8[h][:, :, qcol],
                                     start=True, stop=True, perf_mode=DR)
                    # exp(sim - 4): keeps es inside fp8e4 range (max
                    # finite 240); softmax is shift-invariant
                    if kt % 2 == 0:
                        es_store[(tsq, h, kt // 2)] = esp.tile(
                            [P, 2, TS], f8, tag="es",
                            name=f"es{tsq}_{h}_{kt // 2}")
                    nc.scalar.activation(
                        es_store[(tsq, h, kt // 2)][:, kt % 2, :],
                        psim, AF.Exp, bias=neg2_col)

            with tc.tile_pool(name="xp", bufs=1) as xp:
                # ---- load x^T, ts-chunked so stats/proj of slice 0 start
                # as early as possible ----
                # x in two column-halves and wf in column-quarters,
                # interleaved so that (a) slice-0/1 stats are gated on only
                # half of x and (b) the first proj chains are gated on only
                # the first quarter of wf -> the projection starts ~10us
                # earlier than with monolithic loads
                NKP = NK // 2
                xhi = [xp.tile([P, 2, T], f8, name=f"xhi{kp}")
                       for kp in range(NKP)]
                xlo = [xp.tile([P, 2, T], f8, name=f"xlo{kp}")
                       for kp in range(NKP)]
                whi = [xp.tile([P, 2, FSH], f8, name=f"whi{kp}")
                       for kp in range(NKP)]
                wlo = [xp.tile([P, 2, FSH], f8, name=f"wlo{kp}")
                       for kp in range(NKP)]
                sw_sb = xp.tile([1, 2, FSH], f8)
                nc.sync.dma_start(out=sw_sb, in_=sw_d[:, :, :])
                WQ = FSH // 4
                xh1 = slice(0, T // 2)
                xh2 = slice(T // 2, T)
                for kp in range(NKP):
                    for i in range(2):
                        nc.sync.dma_start(out=xhi[kp][:, i, xh1],
                                          in_=xhi_d[:, kp, i, xh1])
                for q in range(2):
                    qc = slice(q * WQ, (q + 1) * WQ)
                    for kp in range(NKP):
                        nc.sync.dma_start(out=whi[kp][:, :, qc],
                                          in_=whi_d[:, kp, :, qc])
                        nc.sync.dma_start(out=wlo[kp][:, :, qc],
                                          in_=wlo_d[:, kp, :, qc])
                for kp in range(NKP):
                    for i in range(2):
                        nc.sync.dma_start(out=xlo[kp][:, i, xh1],
                                          in_=xlo_d[:, kp, i, xh1])
                for kp in range(NKP):
                    for i in range(2):
                        nc.sync.dma_start(out=xhi[kp][:, i, xh2],
                                          in_=xhi_d[:, kp, i, xh2])
                for q in range(2, 4):
                    qc = slice(q * WQ, (q + 1) * WQ)
                    for kp in range(NKP):
                        nc.sync.dma_start(out=whi[kp][:, :, qc],
                                          in_=whi_d[:, kp, :, qc])
                        nc.sync.dma_start(out=wlo[kp][:, :, qc],
                                          in_=wlo_d[:, kp, :, qc])
                for kp in range(NKP):
                    for i in range(2):
                        nc.sync.dma_start(out=xlo[kp][:, i, xh2],
                                          in_=xlo_d[:, kp, i, xh2])
                # ---- LayerNorm statistics, per token-slice ----
                def emit_stats(ts):
                    col = slice(ts * TS, (ts + 1) * TS)
                    ps_s = ps.tile([1, TS], f32, tag="pp", bufs=5,
                                   name=f"ps_s{ts}")
                    for kp in range(NKP):
                        nc.tensor.matmul(ps_s, lhsT=ones2_f8[:, :, 0:1],
                                         rhs=xhi[kp][:, :, col],
                                         start=(kp == 0),
                                         stop=(kp == NKP - 1),
                                         perf_mode=DR)
                    ps_s2 = ps.tile([1, TS], f32, tag="pp", bufs=5,
                                    name=f"ps_s2{ts}")
                    for kp in range(NKP):
                        x2t = tmp.tile([P, 2, TS], f8, tag="x2t")
                        nc.scalar.activation(x2t, xhi[kp][:, :, col],
                                             AF.Square, bias=zero_col)
                        nc.tensor.matmul(ps_s2, lhsT=ones2_f8[:, :, 0:1],
                                         rhs=x2t,
                                         start=(kp == 0),
                                         stop=(kp == NKP - 1),
                                         perf_mode=DR)
                    # negvarD = (s^2)/D - s2 = -D*var ;  std = sqrt(-negvarD/D
                    # + eps) ;  mu(bf16) = s/D
                    # square on DVE (copy to SBUF + mul): keeps the ACT
                    # Sqrt function table resident instead of thrashing
                    # Square/Sqrt table sets every slice (1.28us per reload)
                    ssq_sb = rows.tile([1, TS], f32, tag="ssq_sb")
                    nc.vector.tensor_copy(ssq_sb, ps_s)
                    ssq = rows.tile([1, TS], f32, tag="ssq")
                    nc.vector.tensor_mul(ssq, ssq_sb, ps_s)
                    mu_bf_r = rows.tile([1, TS], bf16, tag="mu_bf_r")
                    nc.vector.tensor_scalar_mul(mu_bf_r, ps_s, 1.0 / D)
                    negvar = rows.tile([1, TS], f32, tag="negvar")
                    nc.vector.scalar_tensor_tensor(
                        negvar, ssq, 1.0 / D, ps_s2,
                        op0=mybir.AluOpType.mult,
                        op1=mybir.AluOpType.subtract)
                    std = rows.tile([1, TS], f32, tag="std")
                    nc.scalar.activation(std, negvar, AF.Sqrt, bias=eps_t,
                                         scale=-1.0 / D)
                    rstd_r = rows.tile([1, TS],
                                       f32r if ts == 0 else f32,
                                       tag="rstd_r")
                    with nc.allow_low_precision(
                            reason="f32r broadcast operand; ~19-bit "
                                   "mantissa is plenty for rstd"):
                        nc.vector.reciprocal(rstd_r, std)
                    nc.vector.tensor_copy(mu2_row[:, 0, col], mu_bf_r)
                    if ts == 0:
                        # slice 0 gates the whole projection phase: broadcast
                        # via K=1 PE matmul (DMA-free; the DRAM-bounce path
                        # would queue behind the bulk weight loads)
                        prs = ps.tile([P, TS], f32, tag="pp", bufs=5,
                                      name="prs0")
                        nc.tensor.matmul(prs, lhsT=ones_row_r[0:1, :],
                                         rhs=rstd_r,
                                         start=True, stop=True)
                        nc.vector.tensor_copy(rstd_b[:, col], prs)
                    else:
                        # bounce through DRAM, broadcast to 128 partitions
                        # (gpsimd SWDGE queue; lands during the previous
                        # slice's projection)
                        nc.gpsimd.dma_start(out=rstd_d[0:1, col], in_=rstd_r)
                        nc.gpsimd.dma_start(
                            out=rstd_b[:, col],
                            in_=rstd_d[0:1, col].partition_broadcast(P))

                gate = {}

                # ---- fused projection ----
                # packed col order: [q(128) | kv(128) | (gate_j, ffx_j) x 8]
                def emit_proj(ts):
                    col = slice(ts * TS, (ts + 1) * TS)
                    cur_silu = None
                    for fi in range(NF):
                        fcols = slice(fi * P, (fi + 1) * P)
                        pp = ps.tile([P, TS], f32, tag="pp", bufs=5,
                                     name=f"pp{ts}_{fi}")
                        for kp in range(NKP):
                            nc.tensor.matmul(pp, lhsT=whi[kp][:, :, fcols],
                                             rhs=xhi[kp][:, :, col],
                                             start=(kp == 0), stop=False,
                                             perf_mode=DR)
                        # mean correction: accumulate -s_w * mu into the chain
                        nc.tensor.matmul(pp, lhsT=sw_sb[:, :, fcols],
                                         rhs=mu2_row[:, :, col],
                                         start=False, stop=False,
                                         perf_mode=DR)
                        for kp in range(NKP):
                            nc.tensor.matmul(pp, lhsT=whi[kp][:, :, fcols],
                                             rhs=xlo[kp][:, :, col],
                                             start=False, stop=False,
                                             perf_mode=DR)
                        for kp in range(NKP):
                            nc.tensor.matmul(pp, lhsT=wlo[kp][:, :, fcols],
                                             rhs=xhi[kp][:, :, col],
                                             start=False,
                                             stop=(kp == NKP - 1),
                                             perf_mode=DR)
                        if fi == 0:
                            # q evac straight to fp8, then DMA-split the four
                            # 32-partition groups into DoubleRow slot layout
                            q8t = tmp.tile([P, TS], f8, tag="q8t")
                            nc.vector.scalar_tensor_tensor(
                                q8t, pp, 1.0 / 16.0, rstd_b[:, col],
                                op0=mybir.AluOpType.mult,
                                op1=mybir.AluOpType.mult)
                            for h in range(LH):
                                for i in range(2):
                                    lo = h * 64 + i * 32
                                    nc.sync.dma_start(
                                        out=q8[h][:, i, col],
                                        in_=q8t[lo:lo + 32, :])
                        elif fi == 1:
                            nc.vector.scalar_tensor_tensor(
                                kv_sb[:, col], pp, 1.0 / 16.0,
                                rstd_b[:, col],
                                op0=mybir.AluOpType.mult,
                                op1=mybir.AluOpType.mult)
                            k8t = tmp.tile([64, TS], f8, tag="k8t")
                            nc.vector.tensor_copy(k8t, kv_sb[0:64, col])
                            nc.sync.dma_start(out=k8[:, 0, col],
                                              in_=k8t[0:32, :])
                            nc.sync.dma_start(out=k8[:, 1, col],
                                              in_=k8t[32:64, :])
                        elif fi % 2 == 0:  # gate_j
                            g = tmp.tile([P, TS], bf16, tag="g")
                            nc.vector.scalar_tensor_tensor(
                                g, pp, 1.0 / 16.0, rstd_b[:, col],
                                op0=mybir.AluOpType.mult,
                                op1=mybir.AluOpType.mult)
                            silu = tmp.tile([P, TS], bf16, tag="silu")
                            nc.scalar.activation(silu, g, AF.Silu,
                                                 bias=zero_col)
                            cur_silu = silu
                        else:  # ffx_j
                            j = (fi - 3) // 2
                            fx = tmp.tile([P, TS], bf16, tag="fx")
                            nc.vector.tensor_mul(fx, pp, rstd_b[:, col])
                            hmul = nc.vector.tensor_mul(h_sb[j][:, col],
                                                        cur_silu, fx)
                            if ts == 1 and fi == NF - 1:
                                gate["i"] = hmul
                    # v -> token-major (bf16 transpose DMA; fp8 transpose is
                    # unsupported), then a cheap DVE downcast into the
                    # DoubleRow pair tiles
                    for kt in range(ts * (TS // P), (ts + 1) * (TS // P)):
                        nc.sync.dma_start(
                            out=v_aug[kt][:, 0:64],
                            in_=kv_sb[64:128, kt * P:(kt + 1) * P],
                            transpose=True)
                        nc.vector.tensor_copy(v8[kt // 2][:, kt % 2, 0:64],
                                              v_aug[kt][:, 0:64])

                # schedule: only stats(0) ahead of proj(0); later slices'
                # stats (and their row math / broadcasts / centering) hide
                # under the previous slice's projection
                emit_stats(0)
                emit_stats(1)
                emit_proj(0)
                emit_stats(2)
                emit_proj(1)
                emit_stats(3)
                # slices 0-1 evacuated: slot-0 sims over their key tiles can
                # fill ACT during proj(2)
                for kt in range(8):
                    emit_sims_pair(0, kt)
                emit_proj(2)
                for kt in range(8, 12):
                    emit_sims_pair(0, kt)
                # slot-0 AV drains eagerly (v8[m] ready through slice 2),
                # freeing es pairs so the buffer pool stays small
                for m in range(6):
                    av_mm(0, 0, m)
                    av_mm(0, 1, m)
                for kt in range(12):
                    emit_sims_pair(1, kt)
                emit_proj(3)
                for kt in range(12, NKT):
                    emit_sims_pair(0, kt)
                for m in range(6, 8):
                    av_mm(0, 0, m)
                    av_mm(0, 1, m)
                # output-side weights: needed only ~190us in.  Explicitly
                # gated behind the end of proj slice 1 so the scheduler does
                # not hoist these (dependency-free) DMAs ahead of the x/wf
                # loads and halve the effective prologue load bandwidth.
                wao_sb = wp.tile([64, 2, D], f8)
                w_in = nc.gpsimd.dma_start(out=wao_sb, in_=wao_d[:, :])
                add_dep_helper(w_in.ins, gate["i"].ins,
                               reason="defer wao load")
                wfo_sb = []
                for k in range(NK):
                    t_ = wp.tile([P, D], bf16, name=f"wfo{k}")
                    w_in = nc.gpsimd.dma_start(out=t_,
                                               in_=wfo_d[k * P:(k + 1) * P, :])
                    add_dep_helper(w_in.ins, gate["i"].ins,
                                   reason="defer wfo load")
                    wfo_sb.append(t_)

            # xp closed: x/wf tiles are dead, reuse SBUF for attention tiles.
            # Attention pipeline over tsq-slots.  Head-0 sims run in PE rows
            # 0-63, head-1 sims concurrently in rows 64-127 (k replicated at
            # partitions 64-127, q head 1 already there).  AV matmuls of the
            # previous slot and y-chain matmuls interleave at ~exp rate so
            # the PE stays busy while ACT churns the exps.
            with (
                tc.tile_pool(name="atmp", bufs=3) as atmp,
                tc.tile_pool(name="yp", bufs=4) as yp,
                tc.tile_pool(name="yffp", bufs=1) as yffp,
            ):
                y_chains = []

                # Slice 0's output chains are split: the ff-only part runs
                # during attention slots 0-1 (when no other y work is
                # unlocked yet and the PE would otherwise wait on ACT exps),
                # accumulating to SBUF; the single attn matmul merges in
                # during evacuation once slice 0's attention output exists.
                yff_sb = [yffp.tile([P, TS], f32, name=f"yff{d}")
                          for d in range(NK)]

                def y_ff_chain_gen(tsq, d):
                    qcol = slice(tsq * TS, (tsq + 1) * TS)
                    py = ps.tile([P, TS], f32, tag="pp", bufs=5,
                                 name=f"pyf{tsq}_{d}")
                    for k in range(NK):
                        nc.tensor.matmul(
                            py, lhsT=wfo_sb[k][:, d * P:(d + 1) * P],
                            rhs=h_sb[k][:, qcol],
                            start=(k == 0), stop=(k == NK - 1))
                        yield
                    nc.vector.tensor_scalar_mul(yff_sb[d], py, 1.0 / 256.0)

                def y_attn_chain_gen(tsq, d):
                    qcol = slice(tsq * TS, (tsq + 1) * TS)
                    pa = ps.tile([P, TS], f32, tag="pp", bufs=5,
                                 name=f"pya{tsq}_{d}")
                    nc.tensor.matmul(pa, lhsT=wao_sb[:, :, d * P:(d + 1) * P],
                                     rhs=out_sb[:, :, qcol],
                                     start=True, stop=True, perf_mode=DR)
                    yield
                    y_sb = yp.tile([P, TS], f32, tag="ysb",
                                   name=f"ysba{tsq}_{d}")
                    nc.vector.scalar_tensor_tensor(
                        y_sb, pa, 1.0 / 256.0, yff_sb[d],
                        op0=mybir.AluOpType.mult,
                        op1=mybir.AluOpType.add)
                    nc.gpsimd.dma_start(out=yT_d[d * P:(d + 1) * P, qcol],
                                        in_=y_sb)

                def y_chain_gen(tsq, d):
                    qcol = slice(tsq * TS, (tsq + 1) * TS)
                    py = ps.tile([P, TS], f32, tag="pp", bufs=5,
                                 name=f"py{tsq}_{d}")
                    for k in range(NK):
                        nc.tensor.matmul(
                            py, lhsT=wfo_sb[k][:, d * P:(d + 1) * P],
                            rhs=h_sb[k][:, qcol],
                            start=(k == 0), stop=False)
                        yield
                    nc.tensor.matmul(
                        py, lhsT=wao_sb[:, :, d * P:(d + 1) * P],
                        rhs=out_sb[:, :, qcol], start=False, stop=True,
                        perf_mode=DR)
                    y_sb = yp.tile([P, TS], f32, tag="ysb",
                                   name=f"ysb{tsq}_{d}")
                    nc.vector.tensor_scalar_mul(y_sb, py, 1.0 / 256.0)
                    nc.gpsimd.dma_start(out=yT_d[d * P:(d + 1) * P, qcol],
                                        in_=y_sb)

                def y_step(n):
                    done = 0
                    while done < n and y_chains:
                        try:
                            next(y_chains[0])
                        except StopIteration:
                            y_chains.pop(0)
                        done += 1

                def emit_av_epilogue(tsq, h):
                    b = tsq * LH + h
                    qcol = slice(tsq * TS, (tsq + 1) * TS)
                    pav = pavs.pop((tsq, h))
                    # denominator (partition 64) -> reciprocal (stays at
                    # partition 64) -> K=1 fp32 PE broadcast over 64 rows,
                    # reading the stationary+moving operands at partition 64
                    rec64 = atmp.tile([P, TS], mybir.dt.float32r,
                                      tag="rec64")
                    with nc.allow_low_precision(
                            reason="f32r broadcast operand; ~19-bit "
                                   "mantissa is plenty for 1/denom"):
                        nc.vector.reciprocal(rec64[64:65, :],
                                             pav[64:65, :])
                    pB = ps.tile([64, TS], f32, tag="pp", bufs=5,
                                 name=f"pB{b}")
                    nc.tensor.matmul(pB, lhsT=ones_hi[64:65, :],
                                     rhs=rec64[64:65, :],
                                     start=True, stop=True)
                    rb = atmp.tile([64, TS], f32, tag="rb")
                    nc.vector.tensor_copy(rb, pB)
                    # out = 16 * pav * (1/denom); the 16x keeps fp8 values in
                    # the normal range, wao carries the 1/16
                    nc.vector.scalar_tensor_tensor(
                        out_sb[:, h, qcol], pav[0:64, :], 16.0, rb,
                        op0=mybir.AluOpType.mult,
                        op1=mybir.AluOpType.mult)
                    if h == 1:
                        if tsq == 0:
                            # slice-0 attn merges; then slice-1 ff chains
                            # (they reuse the yff tiles slice 0 just drained)
                            y_chains.extend(y_attn_chain_gen(0, d)
                                            for d in range(NK))
                            y_chains.extend(y_ff_chain_gen(1, d)
                                            for d in range(NK))
                        elif tsq == 1:
                            y_chains.extend(y_attn_chain_gen(1, d)
                                            for d in range(NK))
                        else:
                            y_chains.extend(y_chain_gen(tsq, d)
                                            for d in range(NK))

                # drain: es for slots 0-2 is already buffered (sims emitted
                # during proj); remaining sims (rest of slot 1-2 window plus
                # slot 3) interleave with AV + y chains as PE work
                y_chains.extend(y_ff_chain_gen(0, d) for d in range(NK))
                emit_av_epilogue(0, 0)
                emit_av_epilogue(0, 1)
                rest = ([(1, kt) for kt in range(12, NKT)]
                        + [(2, kt) for kt in range(NKT)]
                        + [(3, kt) for kt in range(NKT)])
                ri = 0

                def emit_rest(n):
                    nonlocal ri
                    for _ in range(n):
                        if ri < len(rest):
                            emit_sims_pair(*rest[ri])
                            ri += 1

                for b in range(1, NTS):
                    for m in range(NKT // 2):
                        av_mm(b, 0, m)
                        if m == NKT // 2 - 1:
                            emit_av_epilogue(b, 0)
                        av_mm(b, 1, m)
                        emit_rest(2)
                        y_step(8)
                    emit_av_epilogue(b, 1)
                y_step(1 << 30)

    nc.compile()
    return nc


def _get_nc():
    if "nc" not in _STATE:
        _STATE["nc"] = _build_nc()
    return _STATE["nc"]


def _pack_kp(a):
    """[1024, C] -> [128, 4, 2, C] DoubleRow k-pair layout."""
    c = a.shape[1]
    return np.ascontiguousarray(
        a.reshape(4, 2, P, c).transpose(2, 0, 1, 3))


def _prep_inputs(x, gamma, w_fused, w_attn_out, w_ff_out):
    """Host-side shard packing. Returns in_maps for the 8 cores."""
    x = np.asarray(x, dtype=np.float32)
    gamma = np.asarray(gamma, dtype=np.float32)
    w_fused = np.asarray(w_fused, dtype=np.float32)
    w_attn_out = np.asarray(w_attn_out, dtype=np.float32)
    w_ff_out = np.asarray(w_ff_out, dtype=np.float32)

    # fold gamma into w_fused rows; fold q scale into q columns; 16x so the
    # fp8 residual (w_lo) lands in normal range -- evacs divide it back out
    wf = w_fused * gamma[:, None] * 16.0
    wf = wf.copy()
    wf[:, :ATTN_INNER] *= DH ** -0.5

    q_blk = wf[:, :ATTN_INNER]
    k_blk = wf[:, ATTN_INNER:ATTN_INNER + DH]
    v_blk = wf[:, ATTN_INNER + DH:ATTN_INNER + 2 * DH]
    ffx_blk = wf[:, ATTN_INNER + 2 * DH:ATTN_INNER + 2 * DH + FF_INNER]
    gate_blk = wf[:, ATTN_INNER + 2 * DH + FF_INNER:]

    xhi, xlo = [], []
    for b in range(B):
        xT = np.ascontiguousarray(x[b].T)
        hi = xT.astype(_F8)
        lo = (xT - hi.astype(np.float32)).astype(_F8)
        xhi.append(_pack_kp(hi))
        xlo.append(_pack_kp(lo))

    in_maps = []
    for c in range(NCORES):
        b, s = divmod(c, TP)
        cols = [q_blk[:, P * s:P * s + P], k_blk, v_blk]
        for j in range(NK):
            cols.append(gate_blk[:, FF_SH * s + j * P: FF_SH * s + (j + 1) * P])
            cols.append(ffx_blk[:, FF_SH * s + j * P: FF_SH * s + (j + 1) * P])
        wf_c = np.concatenate(cols, axis=1)
        whi_c = wf_c.astype(_F8)
        wlo_c = (wf_c - whi_c.astype(np.float32)).astype(_F8)
        sw_c = np.zeros((1, 2, FSH), dtype=_F8)
        sw_c[0, 0, :] = (-wf_c.sum(axis=0)).astype(_F8)
        # wao: fp8 DoubleRow layout [64, (head, dcol)], heads in slots.
        # Both out and wao are scaled 16x into fp8e4 normal range; the ff
        # path (fx, wfo) carries the same 16x each, and the y evacuation
        # divides the common 256x back out.
        wao_c = w_attn_out[P * s:P * s + P, :] * 16.0
        wao_c = np.ascontiguousarray(
            np.stack([wao_c[0:64], wao_c[64:128]], axis=1).reshape(64, 2 * D)
        ).astype(_F8)
        wfo_c = np.ascontiguousarray(
            w_ff_out[FF_SH * s:FF_SH * (s + 1), :] * 16.0).astype(_BF16)
        in_maps.append({"xhi": xhi[b], "xlo": xlo[b],
                        "whi": _pack_kp(whi_c), "wlo": _pack_kp(wlo_c),
                        "sw": sw_c, "wao": wao_c, "wfo": wfo_c})
    return in_maps


def kernel(x, gamma, w_fused, w_attn_out, w_ff_out):
    import time
    from concourse.bass_utils import run_bass_kernel_spmd

    nc = _get_nc()
    in_maps = _prep_inputs(x, gamma, w_fused, w_attn_out, w_ff_out)

    t0 = time.perf_counter()
    res = run_bass_kernel_spmd(nc, in_maps, core_ids=list(range(NCORES)))
    t1 = time.perf_counter()
    _STATE["last_wall_ns"] = (t1 - t0) * 1e9

    y = np.empty((B, N, D), dtype=np.float32)
    for b in range(B):
        acc = res.results[b * TP]["yT"].astype(np.float32)
        for s in range(1, TP):
            acc = acc + res.results[b * TP + s]["yT"]
        y[b] = acc.T
    return y



# revision 28
# speedup vs baseline: 1.1036x; 1.1036x over previous
"""Fused transformer block (LN + fused QKV/FF proj + MQA attention + SwiGLU FF)
on 8 TRN2 NeuronCores.

Sharding: hybrid DP2 x TP4.
  core c -> batch b = c//4, tensor-parallel shard s = c%4.
  Each core handles its batch's full 2048 tokens in feature-major layout:
    - q: 2 of 8 heads (cols 128*s .. 128*s+128 of the q block)
    - k/v: replicated (width 64 each)
    - ff: 1024 of 4096 cols of both ff_x and gate
    - attn_out / ff_out: matching row shards -> partial [1024, 2048] outputs
  Host sums the 4 partial outputs per batch (row-parallel reduction).

Device layout is feature-major (features on SBUF partitions, tokens on the
free dim) so every matmul contraction is over the partition dim.  gamma and
the q-scale (dim_head**-0.5) are folded into w_fused on the host; per-token
LayerNorm mu/rstd are computed on device via ones-vector matmuls, broadcast
across partitions through a DRAM bounce, mean-subtract applied in place on
x^T, and rstd folded into each projection's PSUM evacuation.
"""

import numpy as np
import ml_dtypes

# ---- problem shapes (hardcoded) ----
B, N, D = 2, 2048, 1024
DH = 64
HEADS = 8
ATTN_INNER = HEADS * DH          # 512
FF_INNER = 4 * D                 # 4096
T = N                            # tokens per core
P = 128
TS = 512
NTS = T // TS                    # 4
NK = D // P                      # 8
NCORES = 8
TP = 4
LH = HEADS // TP                 # 2 local heads
FF_SH = FF_INNER // TP           # 1024
FSH = LH * DH + 2 * DH + 2 * FF_SH   # 2304 packed proj cols per core
NF = FSH // P                    # 18
NKT = T // P                     # 16 key-token tiles

_BF16 = ml_dtypes.bfloat16
_F8 = ml_dtypes.float8_e4m3

_STATE = {}


def _build_nc():
    from concourse import bacc
    import concourse.tile as tile
    from concourse.tile import add_dep_helper
    import concourse.mybir as mybir

    bf16 = mybir.dt.bfloat16
    f32 = mybir.dt.float32
    f8 = mybir.dt.float8e4
    DR = mybir.MatmulPerfMode.DoubleRow
    AF = mybir.ActivationFunctionType

    nc = bacc.Bacc("TRN2", target_bir_lowering=False, debug=False)

    # x and the fused weights come as 2-level fp8 (hi + residual lo), packed
    # in DoubleRow k-pair layout [p, kp, slot, col]; wf is scaled 16x on the
    # host so the fp8 residual sits in normal range (evac divides by 16)
    xhi_d = nc.dram_tensor("xhi", [P, NK // 2, 2, T], f8, kind="ExternalInput")
    xlo_d = nc.dram_tensor("xlo", [P, NK // 2, 2, T], f8, kind="ExternalInput")
    whi_d = nc.dram_tensor("whi", [P, NK // 2, 2, FSH], f8,
                           kind="ExternalInput")
    wlo_d = nc.dram_tensor("wlo", [P, NK // 2, 2, FSH], f8,
                           kind="ExternalInput")
    sw_d = nc.dram_tensor("sw", [1, 2, FSH], f8, kind="ExternalInput")
    wao_d = nc.dram_tensor("wao", [64, 2 * D], f8, kind="ExternalInput")
    wfo_d = nc.dram_tensor("wfo", [D, D], bf16, kind="ExternalInput")
    yT_d = nc.dram_tensor("yT", [D, T], f32, kind="ExternalOutput")
    # DRAM bounce row for partition-broadcast
    rstd_d = nc.dram_tensor("rstd_bounce", [1, T], f32)

    with tile.TileContext(nc) as tc:
        with (
            tc.tile_pool(name="cp", bufs=1) as cp,
            tc.tile_pool(name="wp", bufs=1) as wp,
            tc.tile_pool(name="acts", bufs=1) as acts,
            tc.tile_pool(name="rows", bufs=1) as rows,
            tc.tile_pool(name="tmp", bufs=3) as tmp,
            tc.tile_pool(name="esp", bufs=20) as esp,
            tc.tile_pool(name="ps", bufs=1, space="PSUM") as ps,
        ):
            # ---- constants ----
            ones_col = cp.tile([P, 1], bf16)
            nc.vector.memset(ones_col, 1.0)
            ones_row_bf = cp.tile([1, P], bf16)
            nc.vector.memset(ones_row_bf, 1.0)
            f32r = mybir.dt.float32r
            ones_hi = cp.tile([P, 64], f32r)
            nc.vector.memset(ones_hi.bitcast(f32), 1.0)
            ones_row_r = cp.tile([1, P], f32r)
            nc.vector.memset(ones_row_r.bitcast(f32), 1.0)
            eps_t = cp.tile([1, 1], f32)
            nc.vector.memset(eps_t, 1e-5)
            zero_col = cp.tile([P, 1], f32)
            nc.vector.memset(zero_col, 0.0)
            neg2_col = cp.tile([P, 1], f32)
            nc.vector.memset(neg2_col, -4.0)
            # DoubleRow lhsT outer stride must be even + 16B aligned
            ones2_f8 = cp.tile([P, 2, 16], f8)
            nc.vector.memset(ones2_f8, 1.0)
            # warm the ACT function tables before the bulk DMAs queue up:
            # lazy table loads otherwise serialize behind ~9MB of weight
            # traffic and stall the first LayerNorm square by ~8us
            warm_o = cp.tile([1, 4], f32)
            for wi, fn in enumerate((AF.Silu, AF.Exp, AF.Sqrt)):
                nc.scalar.activation(warm_o[0:1, wi:wi + 1],
                                     eps_t, fn, bias=zero_col[0:1, :])

            # ---- persistent activations ----
            # q/k in fp8 DoubleRow layout: partition p holds head-dims p and
            # 32+p in slots 0/1 of the middle axis (K = 2x32 per sim matmul)
            q8 = [acts.tile([32, 2, T], f8, name=f"q8_{h}") for h in range(LH)]
            k8 = acts.tile([32, 2, T], f8)
            kv_sb = acts.tile([P, T], bf16)    # k rows 0-63, v rows 64-127
            h_sb = [acts.tile([P, T], bf16, name=f"h{j}") for j in range(NK)]
            # attention out fp8: head h in slot h (partitions 0-63), scaled
            # by 16 to sit in fp8e4 normal range (wao carries the 1/16)
            out_sb = acts.tile([64, 2, T], f8)
            rstd_b = acts.tile([P, T], f32)    # rstd broadcast
            # mu as a 1-row fp8 DoubleRow operand (slot 1 stays zero); the
            # -s_w * mu mean-correction rides the proj PSUM chain, so x is
            # never centered and mu is never partition-broadcast
            mu2_row = acts.tile([1, 2, T], f8)
            nc.vector.memset(mu2_row[:, 1, :], 0.0)
            # v in fp8 DoubleRow pairs: slot i = key-tile 2m+i, col 64 = ones
            # (softmax denominator row)
            v_aug = [acts.tile([P, 72], bf16, name=f"va{kt}") for kt in range(NKT)]
            v8 = [acts.tile([P, 2, 80], f8, name=f"v8_{m}")
                  for m in range(NKT // 2)]
            for kt in range(NKT):
                nc.vector.memset(v_aug[kt][:, 64:65], 1.0)
            for m in range(NKT // 2):
                nc.vector.memset(v8[m][:, 0, 64:65], 1.0)
                nc.vector.memset(v8[m][:, 1, 64:65], 1.0)

            # sims + exp are emitted eagerly during the projection phase
            # (slot b's sims only need slices <= b evacuated): the ACT engine
            # idles during proj and saturates on exp otherwise, so buffering
            # es pairs here converts the ACT-bound attention phase into a
            # short PE-bound drain
            es_store = {}
            pavs = {}

            def av_mm(tsq, h, m):
                if m == 0:
                    pavs[(tsq, h)] = ps.tile([P, TS], f32, tag="pav",
                                             bufs=2, name=f"pav{tsq}_{h}")
                nc.tensor.matmul(
                    pavs[(tsq, h)][0:65, :], lhsT=v8[m][:, :, 0:65],
                    rhs=es_store.pop((tsq, h, m)),
                    start=(m == 0), stop=(m == NKT // 2 - 1),
                    perf_mode=DR)

            def emit_sims_pair(tsq, kt):
                qcol = slice(tsq * TS, (tsq + 1) * TS)
                kcols = slice(kt * P, (kt + 1) * P)
                for h in range(LH):
                    psim = ps.tile([P, TS], f32, tag="psim", bufs=2,
                                   name=f"psim{tsq}_{h}_{kt}")
                    nc.tensor.matmul(psim, lhsT=k8[:, :, kcols],
                                     rhs=q8[h][:, :, qcol],
                                     start=True, stop=True, perf_mode=DR)
                    # exp(sim - 4): keeps es inside fp8e4 range (max
                    # finite 240); softmax is shift-invariant
                    if kt % 2 == 0:
                        es_store[(tsq, h, kt // 2)] = esp.tile(
                            [P, 2, TS], f8, tag="es",
                            name=f"es{tsq}_{h}_{kt // 2}")
                    nc.scalar.activation(
                        es_store[(tsq, h, kt // 2)][:, kt % 2, :],
                        psim, AF.Exp, bias=neg2_col)

            with tc.tile_pool(name="xp", bufs=1) as xp:
                # ---- load x^T, ts-chunked so stats/proj of slice 0 start
                # as early as possible ----
                # x in two column-halves and wf in column-quarters,
                # interleaved so that (a) slice-0/1 stats are gated on only
                # half of x and (b) the first proj chains are gated on only
                # the first quarter of wf -> the projection starts ~10us
                # earlier than with monolithic loads
                NKP = NK // 2
                xhi = [xp.tile([P, 2, T], f8, name=f"xhi{kp}")
                       for kp in range(NKP)]
                xlo = [xp.tile([P, 2, T], f8, name=f"xlo{kp}")
                       for kp in range(NKP)]
                whi = [xp.tile([P, 2, FSH], f8, name=f"whi{kp}")
                       for kp in range(NKP)]
                wlo = [xp.tile([P, 2, FSH], f8, name=f"wlo{kp}")
                       for kp in range(NKP)]
                sw_sb = xp.tile([1, 2, FSH], f8)
                nc.sync.dma_start(out=sw_sb, in_=sw_d[:, :, :])
                WQ = FSH // 4
                xh1 = slice(0, T // 2)
                xh2 = slice(T // 2, T)
                for kp in range(NKP):
                    for i in range(2):
                        nc.sync.dma_start(out=xhi[kp][:, i, xh1],
                                          in_=xhi_d[:, kp, i, xh1])
                for q in range(2):
                    qc = slice(q * WQ, (q + 1) * WQ)
                    for kp in range(NKP):
                        nc.sync.dma_start(out=whi[kp][:, :, qc],
                                          in_=whi_d[:, kp, :, qc])
                        nc.sync.dma_start(out=wlo[kp][:, :, qc],
                                          in_=wlo_d[:, kp, :, qc])
                for kp in range(NKP):
                    for i in range(2):
                        nc.sync.dma_start(out=xlo[kp][:, i, xh1],
                                          in_=xlo_d[:, kp, i, xh1])
                for kp in range(NKP):
                    for i in range(2):
                        nc.sync.dma_start(out=xhi[kp][:, i, xh2],
                                          in_=xhi_d[:, kp, i, xh2])
                for q in range(2, 4):
                    qc = slice(q * WQ, (q + 1) * WQ)
                    for kp in range(NKP):
                        nc.sync.dma_start(out=whi[kp][:, :, qc],
                                          in_=whi_d[:, kp, :, qc])
                        nc.sync.dma_start(out=wlo[kp][:, :, qc],
                                          in_=wlo_d[:, kp, :, qc])
                for kp in range(NKP):
                    for i in range(2):
                        nc.sync.dma_start(out=xlo[kp][:, i, xh2],
                                          in_=xlo_d[:, kp, i, xh2])
                # ---- LayerNorm statistics, per token-slice ----
                def emit_stats(ts):
                    col = slice(ts * TS, (ts + 1) * TS)
                    ps_s = ps.tile([1, TS], f32, tag="pp", bufs=4,
                                   name=f"ps_s{ts}")
                    for kp in range(NKP):
                        nc.tensor.matmul(ps_s, lhsT=ones2_f8[:, :, 0:1],
                                         rhs=xhi[kp][:, :, col],
                                         start=(kp == 0),
                                         stop=(kp == NKP - 1),
                                         perf_mode=DR)
                    ps_s2 = ps.tile([1, TS], f32, tag="pp", bufs=4,
                                    name=f"ps_s2{ts}")
                    for kp in range(NKP):
                        x2t = tmp.tile([P, 2, TS], f8, tag="x2t")
                        nc.scalar.activation(x2t, xhi[kp][:, :, col],
                                             AF.Square, bias=zero_col)
                        nc.tensor.matmul(ps_s2, lhsT=ones2_f8[:, :, 0:1],
                                         rhs=x2t,
                                         start=(kp == 0),
                                         stop=(kp == NKP - 1),
                                         perf_mode=DR)
                    # negvarD = (s^2)/D - s2 = -D*var ;  std = sqrt(-negvarD/D
                    # + eps) ;  mu(bf16) = s/D
                    # square on DVE (copy to SBUF + mul): keeps the ACT
                    # Sqrt function table resident instead of thrashing
                    # Square/Sqrt table sets every slice (1.28us per reload)
                    ssq_sb = rows.tile([1, TS], f32, tag="ssq_sb")
                    nc.vector.tensor_copy(ssq_sb, ps_s)
                    ssq = rows.tile([1, TS], f32, tag="ssq")
                    nc.vector.tensor_mul(ssq, ssq_sb, ps_s)
                    mu_bf_r = rows.tile([1, TS], bf16, tag="mu_bf_r")
                    nc.vector.tensor_scalar_mul(mu_bf_r, ps_s, 1.0 / D)
                    negvar = rows.tile([1, TS], f32, tag="negvar")
                    nc.vector.scalar_tensor_tensor(
                        negvar, ssq, 1.0 / D, ps_s2,
                        op0=mybir.AluOpType.mult,
                        op1=mybir.AluOpType.subtract)
                    std = rows.tile([1, TS], f32, tag="std")
                    nc.scalar.activation(std, negvar, AF.Sqrt, bias=eps_t,
                                         scale=-1.0 / D)
                    rstd_r = rows.tile([1, TS],
                                       f32r if ts == 0 else f32,
                                       tag="rstd_r")
                    with nc.allow_low_precision(
                            reason="f32r broadcast operand; ~19-bit "
                                   "mantissa is plenty for rstd"):
                        nc.vector.reciprocal(rstd_r, std)
                    nc.vector.tensor_copy(mu2_row[:, 0, col], mu_bf_r)
                    if ts == 0:
                        # slice 0 gates the whole projection phase: broadcast
                        # via K=1 PE matmul (DMA-free; the DRAM-bounce path
                        # would queue behind the bulk weight loads)
                        prs = ps.tile([P, TS], f32, tag="pp", bufs=4,
                                      name="prs0")
                        nc.tensor.matmul(prs, lhsT=ones_row_r[0:1, :],
                                         rhs=rstd_r,
                                         start=True, stop=True)
                        nc.vector.tensor_copy(rstd_b[:, col], prs)
                    else:
                        # bounce through DRAM, broadcast to 128 partitions
                        # (gpsimd SWDGE queue; lands during the previous
                        # slice's projection)
                        nc.gpsimd.dma_start(out=rstd_d[0:1, col], in_=rstd_r)
                        nc.gpsimd.dma_start(
                            out=rstd_b[:, col],
                            in_=rstd_d[0:1, col].partition_broadcast(P))

                gate = {}

                # ---- fused projection ----
                # packed col order: [q(128) | kv(128) | (gate_j, ffx_j) x 8]
                def emit_proj(ts):
                    col = slice(ts * TS, (ts + 1) * TS)
                    cur_silu = None
                    for fi in range(NF):
                        fcols = slice(fi * P, (fi + 1) * P)
                        pp = ps.tile([P, TS], f32, tag="pp", bufs=4,
                                     name=f"pp{ts}_{fi}")
                        for kp in range(NKP):
                            nc.tensor.matmul(pp, lhsT=whi[kp][:, :, fcols],
                                             rhs=xhi[kp][:, :, col],
                                             start=(kp == 0), stop=False,
                                             perf_mode=DR)
                        # mean correction: accumulate -s_w * mu into the chain
                        nc.tensor.matmul(pp, lhsT=sw_sb[:, :, fcols],
                                         rhs=mu2_row[:, :, col],
                                         start=False, stop=False,
                                         perf_mode=DR)
                        for kp in range(NKP):
                            nc.tensor.matmul(pp, lhsT=whi[kp][:, :, fcols],
                                             rhs=xlo[kp][:, :, col],
                                             start=False, stop=False,
                                             perf_mode=DR)
                        for kp in range(NKP):
                            nc.tensor.matmul(pp, lhsT=wlo[kp][:, :, fcols],
                                             rhs=xhi[kp][:, :, col],
                                             start=False,
                                             stop=(kp == NKP - 1),
                                             perf_mode=DR)
                        if fi == 0:
                            # q evac straight to fp8, then DMA-split the four
                            # 32-partition groups into DoubleRow slot layout
                            q8t = tmp.tile([P, TS], f8, tag="q8t")
                            nc.vector.scalar_tensor_tensor(
                                q8t, pp, 1.0 / 16.0, rstd_b[:, col],
                                op0=mybir.AluOpType.mult,
                                op1=mybir.AluOpType.mult)
                            for h in range(LH):
                                for i in range(2):
                                    lo = h * 64 + i * 32
                                    nc.sync.dma_start(
                                        out=q8[h][:, i, col],
                                        in_=q8t[lo:lo + 32, :])
                        elif fi == 1:
                            nc.vector.scalar_tensor_tensor(
                                kv_sb[:, col], pp, 1.0 / 16.0,
                                rstd_b[:, col],
                                op0=mybir.AluOpType.mult,
                                op1=mybir.AluOpType.mult)
                            k8t = tmp.tile([64, TS], f8, tag="k8t")
                            nc.vector.tensor_copy(k8t, kv_sb[0:64, col])
                            nc.sync.dma_start(out=k8[:, 0, col],
                                              in_=k8t[0:32, :])
                            nc.sync.dma_start(out=k8[:, 1, col],
                                              in_=k8t[32:64, :])
                        elif fi % 2 == 0:  # gate_j
                            g = tmp.tile([P, TS], bf16, tag="g")
                            nc.vector.scalar_tensor_tensor(
                                g, pp, 1.0 / 16.0, rstd_b[:, col],
                                op0=mybir.AluOpType.mult,
                                op1=mybir.AluOpType.mult)
                            silu = tmp.tile([P, TS], bf16, tag="silu")
                            nc.scalar.activation(silu, g, AF.Silu,
                                                 bias=zero_col)
                            cur_silu = silu
                        else:  # ffx_j
                            j = (fi - 3) // 2
                            fx = tmp.tile([P, TS], bf16, tag="fx")
                            nc.vector.tensor_mul(fx, pp, rstd_b[:, col])
                            hmul = nc.vector.tensor_mul(h_sb[j][:, col],
                                                        cur_silu, fx)
                            if ts == 1 and fi == NF - 1:
                                gate["i"] = hmul
                    # v -> token-major (bf16 transpose DMA; fp8 transpose is
                    # unsupported), then a cheap DVE downcast into the
                    # DoubleRow pair tiles
                    for kt in range(ts * (TS // P), (ts + 1) * (TS // P)):
                        nc.sync.dma_start(
                            out=v_aug[kt][:, 0:64],
                            in_=kv_sb[64:128, kt * P:(kt + 1) * P],
                            transpose=True)
                        nc.vector.tensor_copy(v8[kt // 2][:, kt % 2, 0:64],
                                              v_aug[kt][:, 0:64])

                # schedule: only stats(0) ahead of proj(0); later slices'
                # stats (and their row math / broadcasts / centering) hide
                # under the previous slice's projection
                emit_stats(0)
                emit_stats(1)
                emit_proj(0)
                emit_stats(2)
                emit_proj(1)
                emit_stats(3)
                # slices 0-1 evacuated: slot-0 sims over their key tiles can
                # fill ACT during proj(2)
                for kt in range(8):
                    emit_sims_pair(0, kt)
                emit_proj(2)
                for kt in range(8, 12):
                    emit_sims_pair(0, kt)
                # slot-0 AV drains eagerly (v8[m] ready through slice 2),
                # freeing es pairs so the buffer pool stays small
                for m in range(6):
                    av_mm(0, 0, m)
                    av_mm(0, 1, m)
                for kt in range(12):
                    emit_sims_pair(1, kt)
                emit_proj(3)
                for kt in range(12, NKT):
                    emit_sims_pair(0, kt)
                for m in range(6, 8):
                    av_mm(0, 0, m)
                    av_mm(0, 1, m)
                # output-side weights: needed only ~190us in.  Explicitly
                # gated behind the end of proj slice 1 so the scheduler does
                # not hoist these (dependency-free) DMAs ahead of the x/wf
                # loads and halve the effective prologue load bandwidth.
                wao_sb = wp.tile([64, 2, D], f8)
                w_in = nc.gpsimd.dma_start(out=wao_sb, in_=wao_d[:, :])
                add_dep_helper(w_in.ins, gate["i"].ins,
                               reason="defer wao load")
                wfo_sb = []
                for k in range(NK):
                    t_ = wp.tile([P, D], bf16, name=f"wfo{k}")
                    w_in = nc.gpsimd.dma_start(out=t_,
                                               in_=wfo_d[k * P:(k + 1) * P, :])
                    add_dep_helper(w_in.ins, gate["i"].ins,
                                   reason="defer wfo load")
                    wfo_sb.append(t_)

            # xp closed: x/wf tiles are dead, reuse SBUF for attention tiles.
            # Attention pipeline over tsq-slots.  Head-0 sims run in PE rows
            # 0-63, head-1 sims concurrently in rows 64-127 (k replicated at
            # partitions 64-127, q head 1 already there).  AV matmuls of the
            # previous slot and y-chain matmuls interleave at ~exp rate so
            # the PE stays busy while ACT churns the exps.
            with (
                tc.tile_pool(name="atmp", bufs=3) as atmp,
                tc.tile_pool(name="yp", bufs=4) as yp,
                tc.tile_pool(name="yffp", bufs=1) as yffp,
            ):
                y_chains = []

                # Slice 0's output chains are split: the ff-only part runs
                # during attention slots 0-1 (when no other y work is
                # unlocked yet and the PE would otherwise wait on ACT exps),
                # accumulating to SBUF; the single attn matmul merges in
                # during evacuation once slice 0's attention output exists.
                yff_sb = [yffp.tile([P, TS], f32, name=f"yff{d}")
                          for d in range(NK)]

                def y_ff_chain_gen(tsq, d):
                    qcol = slice(tsq * TS, (tsq + 1) * TS)
                    py = ps.tile([P, TS], f32, tag="pp", bufs=4,
                                 name=f"pyf{tsq}_{d}")
                    for k in range(NK):
                        nc.tensor.matmul(
                            py, lhsT=wfo_sb[k][:, d * P:(d + 1) * P],
                            rhs=h_sb[k][:, qcol],
                            start=(k == 0), stop=(k == NK - 1))
                        yield
                    nc.vector.tensor_scalar_mul(yff_sb[d], py, 1.0 / 256.0)

                def y_attn_chain_gen(tsq, d):
                    qcol = slice(tsq * TS, (tsq + 1) * TS)
                    pa = ps.tile([P, TS], f32, tag="pp", bufs=4,
                                 name=f"pya{tsq}_{d}")
                    nc.tensor.matmul(pa, lhsT=wao_sb[:, :, d * P:(d + 1) * P],
                                     rhs=out_sb[:, :, qcol],
                                     start=True, stop=True, perf_mode=DR)
                    yield
                    y_sb = yp.tile([P, TS], f32, tag="ysb",
                                   name=f"ysba{tsq}_{d}")
                    nc.vector.scalar_tensor_tensor(
                        y_sb, pa, 1.0 / 256.0, yff_sb[d],
                        op0=mybir.AluOpType.mult,
                        op1=mybir.AluOpType.add)
                    nc.gpsimd.dma_start(out=yT_d[d * P:(d + 1) * P, qcol],
                                        in_=y_sb)

                def y_chain_gen(tsq, d):
                    qcol = slice(tsq * TS, (tsq + 1) * TS)
                    py = ps.tile([P, TS], f32, tag="pp", bufs=4,
                                 name=f"py{tsq}_{d}")
                    for k in range(NK):
                        nc.tensor.matmul(
                            py, lhsT=wfo_sb[k][:, d * P:(d + 1) * P],
                            rhs=h_sb[k][:, qcol],
                            start=(k == 0), stop=False)
                        yield
                    nc.tensor.matmul(
                        py, lhsT=wao_sb[:, :, d * P:(d + 1) * P],
                        rhs=out_sb[:, :, qcol], start=False, stop=True,
                        perf_mode=DR)
                    y_sb = yp.tile([P, TS], f32, tag="ysb",
                                   name=f"ysb{tsq}_{d}")
                    nc.vector.tensor_scalar_mul(y_sb, py, 1.0 / 256.0)
                    nc.gpsimd.dma_start(out=yT_d[d * P:(d + 1) * P, qcol],
                                        in_=y_sb)

                def y_step(n):
                    done = 0
                    while done < n and y_chains:
                        try:
                            next(y_chains[0])
                        except StopIteration:
                            y_chains.pop(0)
                        done += 1

                def emit_av_epilogue(tsq, h):
                    b = tsq * LH + h
                    qcol = slice(tsq * TS, (tsq + 1) * TS)
                    pav = pavs.pop((tsq, h))
                    # denominator (partition 64) -> reciprocal (stays at
                    # partition 64) -> K=1 fp32 PE broadcast over 64 rows,
                    # reading the stationary+moving operands at partition 64
                    rec64 = atmp.tile([P, TS], mybir.dt.float32r,
                                      tag="rec64")
                    with nc.allow_low_precision(
                            reason="f32r broadcast operand; ~19-bit "
                                   "mantissa is plenty for 1/denom"):
                        nc.vector.reciprocal(rec64[64:65, :],
                                             pav[64:65, :])
                    pB = ps.tile([64, TS], f32, tag="pp", bufs=4,
                                 name=f"pB{b}")
                    nc.tensor.matmul(pB, lhsT=ones_hi[64:65, :],
                                     rhs=rec64[64:65, :],
                                     start=True, stop=True)
                    rb = atmp.tile([64, TS], f32, tag="rb")
                    nc.vector.tensor_copy(rb, pB)
                    # out = 16 * pav * (1/denom); the 16x keeps fp8 values in
                    # the normal range, wao carries the 1/16
                    nc.vector.scalar_tensor_tensor(
                        out_sb[:, h, qcol], pav[0:64, :], 16.0, rb,
                        op0=mybir.AluOpType.mult,
                        op1=mybir.AluOpType.mult)
                    if h == 1:
                        if tsq == 0:
                            # slice-0 attn merges; then slice-1 ff chains
                            # (they reuse the yff tiles slice 0 just drained)
                            y_chains.extend(y_attn_chain_gen(0, d)
                                            for d in range(NK))
                            y_chains.extend(y_ff_chain_gen(1, d)
                                            for d in range(NK))
                        elif tsq == 1:
                            y_chains.extend(y_attn_chain_gen(1, d)
                                            for d in range(NK))
                        else:
                            y_chains.extend(y_chain_gen(tsq, d)
                                            for d in range(NK))

                # drain: es for slots 0-2 is already buffered (sims emitted
                # during proj); remaining sims (rest of slot 1-2 window plus
                # slot 3) interleave with AV + y chains as PE work
                y_chains.extend(y_ff_chain_gen(0, d) for d in range(NK))
                emit_av_epilogue(0, 0)
                emit_av_epilogue(0, 1)
                rest = ([(1, kt) for kt in range(12, NKT)]
                        + [(2, kt) for kt in range(NKT)]
                        + [(3, kt) for kt in range(NKT)])
                ri = 0

                def emit_rest(n):
                    nonlocal ri
                    for _ in range(n):
                        if ri < len(rest):
                            emit_sims_pair(*rest[ri])
                            ri += 1

                for b in range(1, NTS):
                    for m in range(NKT // 2):
                        av_mm(b, 0, m)
                        if m == NKT // 2 - 1:
                            emit_av_epilogue(b, 0)
                        av_mm(b, 1, m)
                        emit_rest(2)
                        y_step(8)
                    emit_av_epilogue(b, 1)
                y_step(1 << 30)

    nc.compile()
    return nc


def _get_nc():
    if "nc" not in _STATE:
        _STATE["nc"] = _build_nc()
    return _STATE["nc"]


def _pack_kp(a):
    """[1024, C] -> [128, 4, 2, C] DoubleRow k-pair layout."""
    c = a.shape[1]
    return np.ascontiguousarray(
        a.reshape(4, 2, P, c).transpose(2, 0, 1, 3))


def _prep_inputs(x, gamma, w_fused, w_attn_out, w_ff_out):
    """Host-side shard packing. Returns in_maps for the 8 cores."""
    x = np.asarray(x, dtype=np.float32)
    gamma = np.asarray(gamma, dtype=np.float32)
    w_fused = np.asarray(w_fused, dtype=np.float32)
    w_attn_out = np.asarray(w_attn_out, dtype=np.float32)
    w_ff_out = np.asarray(w_ff_out, dtype=np.float32)

    # fold gamma into w_fused rows; fold q scale into q columns; 16x so the
    # fp8 residual (w_lo) lands in normal range -- evacs divide it back out
    wf = w_fused * gamma[:, None] * 16.0
    wf = wf.copy()
    wf[:, :ATTN_INNER] *= DH ** -0.5

    q_blk = wf[:, :ATTN_INNER]
    k_blk = wf[:, ATTN_INNER:ATTN_INNER + DH]
    v_blk = wf[:, ATTN_INNER + DH:ATTN_INNER + 2 * DH]
    ffx_blk = wf[:, ATTN_INNER + 2 * DH:ATTN_INNER + 2 * DH + FF_INNER]
    gate_blk = wf[:, ATTN_INNER + 2 * DH + FF_INNER:]

    xhi, xlo = [], []
    for b in range(B):
        xT = np.ascontiguousarray(x[b].T)
        hi = xT.astype(_F8)
        lo = (xT - hi.astype(np.float32)).astype(_F8)
        xhi.append(_pack_kp(hi))
        xlo.append(_pack_kp(lo))

    in_maps = []
    for c in range(NCORES):
        b, s = divmod(c, TP)
        cols = [q_blk[:, P * s:P * s + P], k_blk, v_blk]
        for j in range(NK):
            cols.append(gate_blk[:, FF_SH * s + j * P: FF_SH * s + (j + 1) * P])
            cols.append(ffx_blk[:, FF_SH * s + j * P: FF_SH * s + (j + 1) * P])
        wf_c = np.concatenate(cols, axis=1)
        whi_c = wf_c.astype(_F8)
        wlo_c = (wf_c - whi_c.astype(np.float32)).astype(_F8)
        sw_c = np.zeros((1, 2, FSH), dtype=_F8)
        sw_c[0, 0, :] = (-wf_c.sum(axis=0)).astype(_F8)
        # wao: fp8 DoubleRow layout [64, (head, dcol)], heads in slots.
        # Both out and wao are scaled 16x into fp8e4 normal range; the ff
        # path (fx, wfo) carries the same 16x each, and the y evacuation
        # divides the common 256x back out.
        wao_c = w_attn_out[P * s:P * s + P, :] * 16.0
        wao_c = np.ascontiguousarray(
            np.stack([wao_c[0:64], wao_c[64:128]], axis=1).reshape(64, 2 * D)
        ).astype(_F8)
        wfo_c = np.ascontiguousarray(
            w_ff_out[FF_SH * s:FF_SH * (s + 1), :] * 16.0).astype(_BF16)
        in_maps.append({"xhi": xhi[b], "xlo": xlo[b],
                        "whi": _pack_kp(whi_c), "wlo": _pack_kp(wlo_c),
                        "sw": sw_c, "wao": wao_c, "wfo": wfo_c})
    return in_maps


def kernel(x, gamma, w_fused, w_attn_out, w_ff_out):
    import time
    from concourse.bass_utils import run_bass_kernel_spmd

    nc = _get_nc()
    in_maps = _prep_inputs(x, gamma, w_fused, w_attn_out, w_ff_out)

    t0 = time.perf_counter()
    res = run_bass_kernel_spmd(nc, in_maps, core_ids=list(range(NCORES)))
    t1 = time.perf_counter()
    _STATE["last_wall_ns"] = (t1 - t0) * 1e9

    y = np.empty((B, N, D), dtype=np.float32)
    for b in range(B):
        acc = res.results[b * TP]["yT"].astype(np.float32)
        for s in range(1, TP):
            acc = acc + res.results[b * TP + s]["yT"]
        y[b] = acc.T
    return y



# revision 29
# speedup vs baseline: 1.1142x; 1.0097x over previous
"""Fused transformer block (LN + fused QKV/FF proj + MQA attention + SwiGLU FF)
on 8 TRN2 NeuronCores.

Sharding: hybrid DP2 x TP4.
  core c -> batch b = c//4, tensor-parallel shard s = c%4.
  Each core handles its batch's full 2048 tokens in feature-major layout:
    - q: 2 of 8 heads (cols 128*s .. 128*s+128 of the q block)
    - k/v: replicated (width 64 each)
    - ff: 1024 of 4096 cols of both ff_x and gate
    - attn_out / ff_out: matching row shards -> partial [1024, 2048] outputs
  Host sums the 4 partial outputs per batch (row-parallel reduction).

Device layout is feature-major (features on SBUF partitions, tokens on the
free dim) so every matmul contraction is over the partition dim.  gamma and
the q-scale (dim_head**-0.5) are folded into w_fused on the host; per-token
LayerNorm mu/rstd are computed on device via ones-vector matmuls, broadcast
across partitions through a DRAM bounce, mean-subtract applied in place on
x^T, and rstd folded into each projection's PSUM evacuation.
"""

import numpy as np
import ml_dtypes

# ---- problem shapes (hardcoded) ----
B, N, D = 2, 2048, 1024
DH = 64
HEADS = 8
ATTN_INNER = HEADS * DH          # 512
FF_INNER = 4 * D                 # 4096
T = N                            # tokens per core
P = 128
TS = 512
NTS = T // TS                    # 4
NK = D // P                      # 8
NCORES = 8
TP = 4
LH = HEADS // TP                 # 2 local heads
FF_SH = FF_INNER // TP           # 1024
FSH = LH * DH + 2 * DH + 2 * FF_SH   # 2304 packed proj cols per core
NF = FSH // P                    # 18
NKT = T // P                     # 16 key-token tiles

_BF16 = ml_dtypes.bfloat16
_F8 = ml_dtypes.float8_e4m3

_STATE = {}


def _build_nc():
    from concourse import bacc
    import concourse.tile as tile
    from concourse.tile import add_dep_helper
    import concourse.mybir as mybir

    bf16 = mybir.dt.bfloat16
    f32 = mybir.dt.float32
    f8 = mybir.dt.float8e4
    DR = mybir.MatmulPerfMode.DoubleRow
    AF = mybir.ActivationFunctionType

    nc = bacc.Bacc("TRN2", target_bir_lowering=False, debug=False)

    # x and the fused weights come as 2-level fp8 (hi + residual lo), packed
    # in DoubleRow k-pair layout [p, kp, slot, col]; wf is scaled 16x on the
    # host so the fp8 residual sits in normal range (evac divides by 16)
    xhi_d = nc.dram_tensor("xhi", [P, NK // 2, 2, T], f8, kind="ExternalInput")
    xlo_d = nc.dram_tensor("xlo", [P, NK // 2, 2, T], f8, kind="ExternalInput")
    whi_d = nc.dram_tensor("whi", [P, NK // 2, 2, FSH], f8,
                           kind="ExternalInput")
    wlo_d = nc.dram_tensor("wlo", [P, NK // 2, 2, FSH], f8,
                           kind="ExternalInput")
    sw_d = nc.dram_tensor("sw", [1, 2, FSH], f8, kind="ExternalInput")
    wao_d = nc.dram_tensor("wao", [64, 2 * D], f8, kind="ExternalInput")
    wfo_d = nc.dram_tensor("wfo", [D, D], bf16, kind="ExternalInput")
    yT_d = nc.dram_tensor("yT", [D, T], f32, kind="ExternalOutput")
    # DRAM bounce row for partition-broadcast
    rstd_d = nc.dram_tensor("rstd_bounce", [1, T], f32)

    with tile.TileContext(nc) as tc:
        with (
            tc.tile_pool(name="cp", bufs=1) as cp,
            tc.tile_pool(name="wp", bufs=1) as wp,
            tc.tile_pool(name="acts", bufs=1) as acts,
            tc.tile_pool(name="rows", bufs=1) as rows,
            tc.tile_pool(name="tmp", bufs=3) as tmp,
            tc.tile_pool(name="esp", bufs=20) as esp,
            tc.tile_pool(name="ps", bufs=1, space="PSUM") as ps,
        ):
            # ---- constants ----
            ones_col = cp.tile([P, 1], bf16)
            nc.vector.memset(ones_col, 1.0)
            ones_row_bf = cp.tile([1, P], bf16)
            nc.vector.memset(ones_row_bf, 1.0)
            f32r = mybir.dt.float32r
            ones_hi = cp.tile([P, 64], f32r)
            nc.vector.memset(ones_hi.bitcast(f32), 1.0)
            ones_row_r = cp.tile([1, P], f32r)
            nc.vector.memset(ones_row_r.bitcast(f32), 1.0)
            eps_t = cp.tile([1, 1], f32)
            nc.vector.memset(eps_t, 1e-5)
            zero_col = cp.tile([P, 1], f32)
            nc.vector.memset(zero_col, 0.0)
            neg2_col = cp.tile([P, 1], f32)
            nc.vector.memset(neg2_col, -4.0)
            # DoubleRow lhsT outer stride must be even + 16B aligned
            ones2_f8 = cp.tile([P, 2, 16], f8)
            nc.vector.memset(ones2_f8, 1.0)
            # warm the ACT function tables before the bulk DMAs queue up:
            # lazy table loads otherwise serialize behind ~9MB of weight
            # traffic and stall the first LayerNorm square by ~8us
            warm_o = cp.tile([1, 4], f32)
            for wi, fn in enumerate((AF.Silu, AF.Exp, AF.Sqrt)):
                nc.scalar.activation(warm_o[0:1, wi:wi + 1],
                                     eps_t, fn, bias=zero_col[0:1, :])

            # ---- persistent activations ----
            # q/k in fp8 DoubleRow layout: partition p holds head-dims p and
            # 32+p in slots 0/1 of the middle axis (K = 2x32 per sim matmul)
            q8 = [acts.tile([32, 2, T], f8, name=f"q8_{h}") for h in range(LH)]
            k8 = acts.tile([32, 2, T], f8)
            kv_sb = acts.tile([P, T], bf16)    # k rows 0-63, v rows 64-127
            h_sb = [acts.tile([P, T], bf16, name=f"h{j}") for j in range(NK)]
            # attention out fp8: head h in slot h (partitions 0-63), scaled
            # by 16 to sit in fp8e4 normal range (wao carries the 1/16)
            out_sb = acts.tile([64, 2, T], f8)
            rstd_b = acts.tile([P, T], f32)    # rstd broadcast
            # mu as a 1-row fp8 DoubleRow operand (slot 1 stays zero); the
            # -s_w * mu mean-correction rides the proj PSUM chain, so x is
            # never centered and mu is never partition-broadcast
            mu2_row = acts.tile([1, 2, T], f8)
            nc.vector.memset(mu2_row[:, 1, :], 0.0)
            # v in fp8 DoubleRow pairs: slot i = key-tile 2m+i, col 64 = ones
            # (softmax denominator row)
            v_aug = [acts.tile([P, 72], bf16, name=f"va{kt}") for kt in range(NKT)]
            v8 = [acts.tile([P, 2, 80], f8, name=f"v8_{m}")
                  for m in range(NKT // 2)]
            for kt in range(NKT):
                nc.vector.memset(v_aug[kt][:, 64:65], 1.0)
            for m in range(NKT // 2):
                nc.vector.memset(v8[m][:, 0, 64:65], 1.0)
                nc.vector.memset(v8[m][:, 1, 64:65], 1.0)

            # sims + exp are emitted eagerly during the projection phase
            # (slot b's sims only need slices <= b evacuated): the ACT engine
            # idles during proj and saturates on exp otherwise, so buffering
            # es pairs here converts the ACT-bound attention phase into a
            # short PE-bound drain
            es_store = {}
            pavs = {}

            def av_mm(tsq, h, m):
                if m == 0:
                    pavs[(tsq, h)] = ps.tile([P, TS], f32, tag="pav",
                                             bufs=2, name=f"pav{tsq}_{h}")
                nc.tensor.matmul(
                    pavs[(tsq, h)][0:65, :], lhsT=v8[m][:, :, 0:65],
                    rhs=es_store.pop((tsq, h, m)),
                    start=(m == 0), stop=(m == NKT // 2 - 1),
                    perf_mode=DR)

            def emit_sims_pair(tsq, kt):
                qcol = slice(tsq * TS, (tsq + 1) * TS)
                kcols = slice(kt * P, (kt + 1) * P)
                for h in range(LH):
                    psim = ps.tile([P, TS], f32, tag="psim", bufs=2,
                                   name=f"psim{tsq}_{h}_{kt}")
                    nc.tensor.matmul(psim, lhsT=k8[:, :, kcols],
                                     rhs=q8[h][:, :, qcol],
                                     start=True, stop=True, perf_mode=DR)
                    # exp(sim - 4): keeps es inside fp8e4 range (max
                    # finite 240); softmax is shift-invariant
                    if kt % 2 == 0:
                        es_store[(tsq, h, kt // 2)] = esp.tile(
                            [P, 2, TS], f8, tag="es",
                            name=f"es{tsq}_{h}_{kt // 2}")
                    nc.scalar.activation(
                        es_store[(tsq, h, kt // 2)][:, kt % 2, :],
                        psim, AF.Exp, bias=neg2_col)

            with tc.tile_pool(name="xp", bufs=1) as xp:
                # ---- load x^T, ts-chunked so stats/proj of slice 0 start
                # as early as possible ----
                # x in two column-halves and wf in column-quarters,
                # interleaved so that (a) slice-0/1 stats are gated on only
                # half of x and (b) the first proj chains are gated on only
                # the first quarter of wf -> the projection starts ~10us
                # earlier than with monolithic loads
                NKP = NK // 2
                xhi = [xp.tile([P, 2, T], f8, name=f"xhi{kp}")
                       for kp in range(NKP)]
                xlo = [xp.tile([P, 2, T], f8, name=f"xlo{kp}")
                       for kp in range(NKP)]
                whi = [xp.tile([P, 2, FSH], f8, name=f"whi{kp}")
                       for kp in range(NKP)]
                wlo = [xp.tile([P, 2, FSH], f8, name=f"wlo{kp}")
                       for kp in range(NKP)]
                sw_sb = xp.tile([1, 2, FSH], f8)
                nc.sync.dma_start(out=sw_sb, in_=sw_d[:, :, :])
                WQ = FSH // 4
                xh1 = slice(0, T // 2)
                xh2 = slice(T // 2, T)
                for kp in range(NKP):
                    for i in range(2):
                        nc.sync.dma_start(out=xhi[kp][:, i, xh1],
                                          in_=xhi_d[:, kp, i, xh1])
                for q in range(2):
                    qc = slice(q * WQ, (q + 1) * WQ)
                    for kp in range(NKP):
                        nc.sync.dma_start(out=whi[kp][:, :, qc],
                                          in_=whi_d[:, kp, :, qc])
                        nc.sync.dma_start(out=wlo[kp][:, :, qc],
                                          in_=wlo_d[:, kp, :, qc])
                for kp in range(NKP):
                    for i in range(2):
                        nc.sync.dma_start(out=xlo[kp][:, i, xh1],
                                          in_=xlo_d[:, kp, i, xh1])
                for kp in range(NKP):
                    for i in range(2):
                        nc.sync.dma_start(out=xhi[kp][:, i, xh2],
                                          in_=xhi_d[:, kp, i, xh2])
                for q in range(2, 4):
                    qc = slice(q * WQ, (q + 1) * WQ)
                    for kp in range(NKP):
                        nc.sync.dma_start(out=whi[kp][:, :, qc],
                                          in_=whi_d[:, kp, :, qc])
                        nc.sync.dma_start(out=wlo[kp][:, :, qc],
                                          in_=wlo_d[:, kp, :, qc])
                for kp in range(NKP):
                    for i in range(2):
                        nc.sync.dma_start(out=xlo[kp][:, i, xh2],
                                          in_=xlo_d[:, kp, i, xh2])
                # ---- LayerNorm statistics, per token-slice ----
                def emit_stats(ts):
                    col = slice(ts * TS, (ts + 1) * TS)
                    ps_s = ps.tile([1, TS], f32, tag="pp", bufs=4,
                                   name=f"ps_s{ts}")
                    for kp in range(NKP):
                        nc.tensor.matmul(ps_s, lhsT=ones2_f8[:, :, 0:1],
                                         rhs=xhi[kp][:, :, col],
                                         start=(kp == 0),
                                         stop=(kp == NKP - 1),
                                         perf_mode=DR)
                    ps_s2 = ps.tile([1, TS], f32, tag="pp", bufs=4,
                                    name=f"ps_s2{ts}")
                    for kp in range(NKP):
                        x2t = tmp.tile([P, 2, TS], f8, tag="x2t")
                        nc.vector.tensor_mul(x2t, xhi[kp][:, :, col],
                                             xhi[kp][:, :, col])
                        nc.tensor.matmul(ps_s2, lhsT=ones2_f8[:, :, 0:1],
                                         rhs=x2t,
                                         start=(kp == 0),
                                         stop=(kp == NKP - 1),
                                         perf_mode=DR)
                    # negvarD = (s^2)/D - s2 = -D*var ;  std = sqrt(-negvarD/D
                    # + eps) ;  mu(bf16) = s/D
                    # square on DVE (copy to SBUF + mul): keeps the ACT
                    # Sqrt function table resident instead of thrashing
                    # Square/Sqrt table sets every slice (1.28us per reload)
                    ssq_sb = rows.tile([1, TS], f32, tag="ssq_sb")
                    nc.vector.tensor_copy(ssq_sb, ps_s)
                    ssq = rows.tile([1, TS], f32, tag="ssq")
                    nc.vector.tensor_mul(ssq, ssq_sb, ps_s)
                    mu_bf_r = rows.tile([1, TS], bf16, tag="mu_bf_r")
                    nc.vector.tensor_scalar_mul(mu_bf_r, ps_s, 1.0 / D)
                    negvar = rows.tile([1, TS], f32, tag="negvar")
                    nc.vector.scalar_tensor_tensor(
                        negvar, ssq, 1.0 / D, ps_s2,
                        op0=mybir.AluOpType.mult,
                        op1=mybir.AluOpType.subtract)
                    std = rows.tile([1, TS], f32, tag="std")
                    nc.scalar.activation(std, negvar, AF.Sqrt, bias=eps_t,
                                         scale=-1.0 / D)
                    rstd_r = rows.tile([1, TS],
                                       f32r if ts == 0 else f32,
                                       tag="rstd_r")
                    with nc.allow_low_precision(
                            reason="f32r broadcast operand; ~19-bit "
                                   "mantissa is plenty for rstd"):
                        nc.vector.reciprocal(rstd_r, std)
                    nc.vector.tensor_copy(mu2_row[:, 0, col], mu_bf_r)
                    if ts == 0:
                        # slice 0 gates the whole projection phase: broadcast
                        # via K=1 PE matmul (DMA-free; the DRAM-bounce path
                        # would queue behind the bulk weight loads)
                        prs = ps.tile([P, TS], f32, tag="pp", bufs=4,
                                      name="prs0")
                        nc.tensor.matmul(prs, lhsT=ones_row_r[0:1, :],
                                         rhs=rstd_r,
                                         start=True, stop=True)
                        nc.vector.tensor_copy(rstd_b[:, col], prs)
                    else:
                        # bounce through DRAM, broadcast to 128 partitions
                        # (gpsimd SWDGE queue; lands during the previous
                        # slice's projection)
                        nc.gpsimd.dma_start(out=rstd_d[0:1, col], in_=rstd_r)
                        nc.gpsimd.dma_start(
                            out=rstd_b[:, col],
                            in_=rstd_d[0:1, col].partition_broadcast(P))

                gate = {}

                # ---- fused projection ----
                # packed col order: [q(128) | kv(128) | (gate_j, ffx_j) x 8]
                def emit_proj(ts):
                    col = slice(ts * TS, (ts + 1) * TS)
                    cur_silu = None
                    for fi in range(NF):
                        fcols = slice(fi * P, (fi + 1) * P)
                        pp = ps.tile([P, TS], f32, tag="pp", bufs=4,
                                     name=f"pp{ts}_{fi}")
                        for kp in range(NKP):
                            nc.tensor.matmul(pp, lhsT=whi[kp][:, :, fcols],
                                             rhs=xhi[kp][:, :, col],
                                             start=(kp == 0), stop=False,
                                             perf_mode=DR)
                        # mean correction: accumulate -s_w * mu into the chain
                        nc.tensor.matmul(pp, lhsT=sw_sb[:, :, fcols],
                                         rhs=mu2_row[:, :, col],
                                         start=False, stop=False,
                                         perf_mode=DR)
                        for kp in range(NKP):
                            nc.tensor.matmul(pp, lhsT=whi[kp][:, :, fcols],
                                             rhs=xlo[kp][:, :, col],
                                             start=False, stop=False,
                                             perf_mode=DR)
                        for kp in range(NKP):
                            nc.tensor.matmul(pp, lhsT=wlo[kp][:, :, fcols],
                                             rhs=xhi[kp][:, :, col],
                                             start=False,
                                             stop=(kp == NKP - 1),
                                             perf_mode=DR)
                        if fi == 0:
                            # q evac straight to fp8, then DMA-split the four
                            # 32-partition groups into DoubleRow slot layout
                            q8t = tmp.tile([P, TS], f8, tag="q8t")
                            nc.vector.scalar_tensor_tensor(
                                q8t, pp, 1.0 / 16.0, rstd_b[:, col],
                                op0=mybir.AluOpType.mult,
                                op1=mybir.AluOpType.mult)
                            for h in range(LH):
                                for i in range(2):
                                    lo = h * 64 + i * 32
                                    nc.sync.dma_start(
                                        out=q8[h][:, i, col],
                                        in_=q8t[lo:lo + 32, :])
                        elif fi == 1:
                            nc.vector.scalar_tensor_tensor(
                                kv_sb[:, col], pp, 1.0 / 16.0,
                                rstd_b[:, col],
                                op0=mybir.AluOpType.mult,
                                op1=mybir.AluOpType.mult)
                            k8t = tmp.tile([64, TS], f8, tag="k8t")
                            nc.vector.tensor_copy(k8t, kv_sb[0:64, col])
                            nc.sync.dma_start(out=k8[:, 0, col],
                                              in_=k8t[0:32, :])
                            nc.sync.dma_start(out=k8[:, 1, col],
                                              in_=k8t[32:64, :])
                        elif fi % 2 == 0:  # gate_j
                            g = tmp.tile([P, TS], bf16, tag="g")
                            nc.vector.scalar_tensor_tensor(
                                g, pp, 1.0 / 16.0, rstd_b[:, col],
                                op0=mybir.AluOpType.mult,
                                op1=mybir.AluOpType.mult)
                            silu = tmp.tile([P, TS], bf16, tag="silu")
                            nc.scalar.activation(silu, g, AF.Silu,
                                                 bias=zero_col)
                            cur_silu = silu
                        else:  # ffx_j
                            j = (fi - 3) // 2
                            fx = tmp.tile([P, TS], bf16, tag="fx")
                            nc.vector.tensor_mul(fx, pp, rstd_b[:, col])
                            hmul = nc.vector.tensor_mul(h_sb[j][:, col],
                                                        cur_silu, fx)
                            if ts == 1 and fi == NF - 1:
                                gate["i"] = hmul
                    # v -> token-major (bf16 transpose DMA; fp8 transpose is
                    # unsupported), then a cheap DVE downcast into the
                    # DoubleRow pair tiles
                    for kt in range(ts * (TS // P), (ts + 1) * (TS // P)):
                        nc.sync.dma_start(
                            out=v_aug[kt][:, 0:64],
                            in_=kv_sb[64:128, kt * P:(kt + 1) * P],
                            transpose=True)
                        nc.vector.tensor_copy(v8[kt // 2][:, kt % 2, 0:64],
                                              v_aug[kt][:, 0:64])

                # schedule: only stats(0) ahead of proj(0); later slices'
                # stats (and their row math / broadcasts / centering) hide
                # under the previous slice's projection
                emit_stats(0)
                emit_stats(1)
                emit_proj(0)
                emit_stats(2)
                emit_proj(1)
                emit_stats(3)
                # slices 0-1 evacuated: slot-0 sims over their key tiles can
                # fill ACT during proj(2)
                for kt in range(8):
                    emit_sims_pair(0, kt)
                emit_proj(2)
                for kt in range(8, 12):
                    emit_sims_pair(0, kt)
                # slot-0 AV drains eagerly (v8[m] ready through slice 2),
                # freeing es pairs so the buffer pool stays small
                for m in range(6):
                    av_mm(0, 0, m)
                    av_mm(0, 1, m)
                for kt in range(12):
                    emit_sims_pair(1, kt)
                emit_proj(3)
                for kt in range(12, NKT):
                    emit_sims_pair(0, kt)
                for m in range(6, 8):
                    av_mm(0, 0, m)
                    av_mm(0, 1, m)
                # output-side weights: needed only ~190us in.  Explicitly
                # gated behind the end of proj slice 1 so the scheduler does
                # not hoist these (dependency-free) DMAs ahead of the x/wf
                # loads and halve the effective prologue load bandwidth.
                wao_sb = wp.tile([64, 2, D], f8)
                w_in = nc.gpsimd.dma_start(out=wao_sb, in_=wao_d[:, :])
                add_dep_helper(w_in.ins, gate["i"].ins,
                               reason="defer wao load")
                wfo_sb = []
                for k in range(NK):
                    t_ = wp.tile([P, D], bf16, name=f"wfo{k}")
                    w_in = nc.gpsimd.dma_start(out=t_,
                                               in_=wfo_d[k * P:(k + 1) * P, :])
                    add_dep_helper(w_in.ins, gate["i"].ins,
                                   reason="defer wfo load")
                    wfo_sb.append(t_)

            # xp closed: x/wf tiles are dead, reuse SBUF for attention tiles.
            # Attention pipeline over tsq-slots.  Head-0 sims run in PE rows
            # 0-63, head-1 sims concurrently in rows 64-127 (k replicated at
            # partitions 64-127, q head 1 already there).  AV matmuls of the
            # previous slot and y-chain matmuls interleave at ~exp rate so
            # the PE stays busy while ACT churns the exps.
            with (
                tc.tile_pool(name="atmp", bufs=3) as atmp,
                tc.tile_pool(name="yp", bufs=4) as yp,
                tc.tile_pool(name="yffp", bufs=1) as yffp,
            ):
                y_chains = []

                # Slice 0's output chains are split: the ff-only part runs
                # during attention slots 0-1 (when no other y work is
                # unlocked yet and the PE would otherwise wait on ACT exps),
                # accumulating to SBUF; the single attn matmul merges in
                # during evacuation once slice 0's attention output exists.
                yff_sb = [yffp.tile([P, TS], f32, name=f"yff{d}")
                          for d in range(NK)]

                def y_ff_chain_gen(tsq, d):
                    qcol = slice(tsq * TS, (tsq + 1) * TS)
                    py = ps.tile([P, TS], f32, tag="pp", bufs=4,
                                 name=f"pyf{tsq}_{d}")
                    for k in range(NK):
                        nc.tensor.matmul(
                            py, lhsT=wfo_sb[k][:, d * P:(d + 1) * P],
                            rhs=h_sb[k][:, qcol],
                            start=(k == 0), stop=(k == NK - 1))
                        yield
                    nc.vector.tensor_scalar_mul(yff_sb[d], py, 1.0 / 256.0)

                def y_attn_chain_gen(tsq, d):
                    qcol = slice(tsq * TS, (tsq + 1) * TS)
                    pa = ps.tile([P, TS], f32, tag="pp", bufs=4,
                                 name=f"pya{tsq}_{d}")
                    nc.tensor.matmul(pa, lhsT=wao_sb[:, :, d * P:(d + 1) * P],
                                     rhs=out_sb[:, :, qcol],
                                     start=True, stop=True, perf_mode=DR)
                    yield
                    y_sb = yp.tile([P, TS], f32, tag="ysb",
                                   name=f"ysba{tsq}_{d}")
                    nc.vector.scalar_tensor_tensor(
                        y_sb, pa, 1.0 / 256.0, yff_sb[d],
                        op0=mybir.AluOpType.mult,
                        op1=mybir.AluOpType.add)
                    nc.gpsimd.dma_start(out=yT_d[d * P:(d + 1) * P, qcol],
                                        in_=y_sb)

                def y_chain_gen(tsq, d):
                    qcol = slice(tsq * TS, (tsq + 1) * TS)
                    py = ps.tile([P, TS], f32, tag="pp", bufs=4,
                                 name=f"py{tsq}_{d}")
                    for k in range(NK):
                        nc.tensor.matmul(
                            py, lhsT=wfo_sb[k][:, d * P:(d + 1) * P],
                            rhs=h_sb[k][:, qcol],
                            start=(k == 0), stop=False)
                        yield
                    nc.tensor.matmul(
                        py, lhsT=wao_sb[:, :, d * P:(d + 1) * P],
                        rhs=out_sb[:, :, qcol], start=False, stop=True,
                        perf_mode=DR)
                    y_sb = yp.tile([P, TS], f32, tag="ysb",
                                   name=f"ysb{tsq}_{d}")
                    nc.vector.tensor_scalar_mul(y_sb, py, 1.0 / 256.0)
                    nc.gpsimd.dma_start(out=yT_d[d * P:(d + 1) * P, qcol],
                                        in_=y_sb)

                def y_step(n):
                    done = 0
                    while done < n and y_chains:
                        try:
                            next(y_chains[0])
                        except StopIteration:
                            y_chains.pop(0)
                        done += 1

                def emit_av_epilogue(tsq, h):
                    b = tsq * LH + h
                    qcol = slice(tsq * TS, (tsq + 1) * TS)
                    pav = pavs.pop((tsq, h))
                    # denominator (partition 64) -> reciprocal (stays at
                    # partition 64) -> K=1 fp32 PE broadcast over 64 rows,
                    # reading the stationary+moving operands at partition 64
                    rec64 = atmp.tile([P, TS], mybir.dt.float32r,
                                      tag="rec64")
                    with nc.allow_low_precision(
                            reason="f32r broadcast operand; ~19-bit "
                                   "mantissa is plenty for 1/denom"):
                        nc.vector.reciprocal(rec64[64:65, :],
                                             pav[64:65, :])
                    pB = ps.tile([64, TS], f32, tag="pp", bufs=4,
                                 name=f"pB{b}")
                    nc.tensor.matmul(pB, lhsT=ones_hi[64:65, :],
                                     rhs=rec64[64:65, :],
                                     start=True, stop=True)
                    rb = atmp.tile([64, TS], f32, tag="rb")
                    nc.vector.tensor_copy(rb, pB)
                    # out = 16 * pav * (1/denom); the 16x keeps fp8 values in
                    # the normal range, wao carries the 1/16
                    nc.vector.scalar_tensor_tensor(
                        out_sb[:, h, qcol], pav[0:64, :], 16.0, rb,
                        op0=mybir.AluOpType.mult,
                        op1=mybir.AluOpType.mult)
                    if h == 1:
                        if tsq == 0:
                            # slice-0 attn merges; then slice-1 ff chains
                            # (they reuse the yff tiles slice 0 just drained)
                            y_chains.extend(y_attn_chain_gen(0, d)
                                            for d in range(NK))
                            y_chains.extend(y_ff_chain_gen(1, d)
                                            for d in range(NK))
                        elif tsq == 1:
                            y_chains.extend(y_attn_chain_gen(1, d)
                                            for d in range(NK))
                        else:
                            y_chains.extend(y_chain_gen(tsq, d)
                                            for d in range(NK))

                # drain: es for slots 0-2 is already buffered (sims emitted
                # during proj); remaining sims (rest of slot 1-2 window plus
                # slot 3) interleave with AV + y chains as PE work
                y_chains.extend(y_ff_chain_gen(0, d) for d in range(NK))
                emit_av_epilogue(0, 0)
                emit_av_epilogue(0, 1)
                rest = ([(1, kt) for kt in range(12, NKT)]
                        + [(2, kt) for kt in range(NKT)]
                        + [(3, kt) for kt in range(NKT)])
                ri = 0

                def emit_rest(n):
                    nonlocal ri
                    for _ in range(n):
                        if ri < len(rest):
                            emit_sims_pair(*rest[ri])
                            ri += 1

                for b in range(1, NTS):
                    for m in range(NKT // 2):
                        av_mm(b, 0, m)
                        if m == NKT // 2 - 1:
                            emit_av_epilogue(b, 0)
                        av_mm(b, 1, m)
                        emit_rest(2)
                        y_step(8)
                    emit_av_epilogue(b, 1)
                y_step(1 << 30)

    nc.compile()
    return nc


def _get_nc():
    if "nc" not in _STATE:
        _STATE["nc"] = _build_nc()
    return _STATE["nc"]


def _pack_kp(a):
    """[1024, C] -> [128, 4, 2, C] DoubleRow k-pair layout."""
    c = a.shape[1]
    return np.ascontiguousarray(
        a.reshape(4, 2, P, c).transpose(2, 0, 1, 3))


def _prep_inputs(x, gamma, w_fused, w_attn_out, w_ff_out):
    """Host-side shard packing. Returns in_maps for the 8 cores."""
    x = np.asarray(x, dtype=np.float32)
    gamma = np.asarray(gamma, dtype=np.float32)
    w_fused = np.asarray(w_fused, dtype=np.float32)
    w_attn_out = np.asarray(w_attn_out, dtype=np.float32)
    w_ff_out = np.asarray(w_ff_out, dtype=np.float32)

    # fold gamma into w_fused rows; fold q scale into q columns; 16x so the
    # fp8 residual (w_lo) lands in normal range -- evacs divide it back out
    wf = w_fused * gamma[:, None] * 16.0
    wf = wf.copy()
    wf[:, :ATTN_INNER] *= DH ** -0.5

    q_blk = wf[:, :ATTN_INNER]
    k_blk = wf[:, ATTN_INNER:ATTN_INNER + DH]
    v_blk = wf[:, ATTN_INNER + DH:ATTN_INNER + 2 * DH]
    ffx_blk = wf[:, ATTN_INNER + 2 * DH:ATTN_INNER + 2 * DH + FF_INNER]
    gate_blk = wf[:, ATTN_INNER + 2 * DH + FF_INNER:]

    xhi, xlo = [], []
    for b in range(B):
        xT = np.ascontiguousarray(x[b].T)
        hi = xT.astype(_F8)
        lo = (xT - hi.astype(np.float32)).astype(_F8)
        xhi.append(_pack_kp(hi))
        xlo.append(_pack_kp(lo))

    in_maps = []
    for c in range(NCORES):
        b, s = divmod(c, TP)
        cols = [q_blk[:, P * s:P * s + P], k_blk, v_blk]
        for j in range(NK):
            cols.append(gate_blk[:, FF_SH * s + j * P: FF_SH * s + (j + 1) * P])
            cols.append(ffx_blk[:, FF_SH * s + j * P: FF_SH * s + (j + 1) * P])
        wf_c = np.concatenate(cols, axis=1)
        whi_c = wf_c.astype(_F8)
        wlo_c = (wf_c - whi_c.astype(np.float32)).astype(_F8)
        sw_c = np.zeros((1, 2, FSH), dtype=_F8)
        sw_c[0, 0, :] = (-wf_c.sum(axis=0)).astype(_F8)
        # wao: fp8 DoubleRow layout [64, (head, dcol)], heads in slots.
        # Both out and wao are scaled 16x into fp8e4 normal range; the ff
        # path (fx, wfo) carries the same 16x each, and the y evacuation
        # divides the common 256x back out.
        wao_c = w_attn_out[P * s:P * s + P, :] * 16.0
        wao_c = np.ascontiguousarray(
            np.stack([wao_c[0:64], wao_c[64:128]], axis=1).reshape(64, 2 * D)
        ).astype(_F8)
        wfo_c = np.ascontiguousarray(
            w_ff_out[FF_SH * s:FF_SH * (s + 1), :] * 16.0).astype(_BF16)
        in_maps.append({"xhi": xhi[b], "xlo": xlo[b],
                        "whi": _pack_kp(whi_c), "wlo": _pack_kp(wlo_c),
                        "sw": sw_c, "wao": wao_c, "wfo": wfo_c})
    return in_maps


def kernel(x, gamma, w_fused, w_attn_out, w_ff_out):
    import time
    from concourse.bass_utils import run_bass_kernel_spmd

    nc = _get_nc()
    in_maps = _prep_inputs(x, gamma, w_fused, w_attn_out, w_ff_out)

    t0 = time.perf_counter()
    res = run_bass_kernel_spmd(nc, in_maps, core_ids=list(range(NCORES)))
    t1 = time.perf_counter()
    _STATE["last_wall_ns"] = (t1 - t0) * 1e9

    y = np.empty((B, N, D), dtype=np.float32)
    for b in range(B):
        acc = res.results[b * TP]["yT"].astype(np.float32)
        for s in range(1, TP):
            acc = acc + res.results[b * TP + s]["yT"]
        y[b] = acc.T
    return y

